# revision 1
# baseline (speedup 1.0000x reference)
"""GAT (2-layer, 2-head, global-softmax) Trainium2 kernel over 8 NeuronCores.

Strategy (see spec sharding_hint): nodes are partitioned by destination across
the 8 cores (6250 dst nodes each); edges live with their dst owner. The
nonstandard GLOBAL softmax factorizes: out = [sum_e exp(s_e) h_src] / gsum with
gsum a per-head global scalar, so each layer is ONE pass over edges plus a tiny
AllReduce. (The reference's max-subtraction only guards a 1e-10 epsilon that is
~1e-15 relative here, so it is dropped.)

Per layer:
  1. table build (replicated per core): h = act @ W.T in bf16 plus per-node
     scores asrc = h . att_src packed into 512-B rows of a [N, 256] bf16 table;
     a small own-nodes pass writes adst = h . att_dst into a local f32 table.
  2. edge pass over 50 static "supertiles" (125 consecutive dst nodes, padded
     edge capacity): dma_gather of table rows by src (split into two 25000-row
     halves to fit int16 indices; pads point at row 0), dma_gather of adst by
     local dst (pads at a -1e9 sentinel row so exp -> 0), score = lrelu(asrc +
     adst) on ACT, w = exp(score) with accum_out accumulating gsum, messages =
     h * w (broadcast), one-hot(slot) bf16 matmul accumulates the segment sum
     into PSUM [128 slots, 128].
  3. AllReduce gsum; fixup out = (U0/T0 + U1/T1)/2 + b; layer 1 applies relu
     and AllGathers the bf16 activations for the next layer's table build.
"""
import sys

sys.path.insert(0, "/opt/trn_rl_repo")

import numpy as np

N = 50000
FIN = 128
C = 64
H = 2
HC = H * C  # 128
E = 800000
N_CORES = 8
NLOC = N // N_CORES          # 6250
NST_NODES = 125              # dst nodes per supertile
S = NLOC // NST_NODES        # 50 supertiles per core
S_GLOBAL = S * N_CORES       # 400
V_HALF = N // 2              # 25000 rows per table half (int16-safe indices)
TROW = 256                   # bf16 elems per table row (512 B): h[0:128], asrc[128:130]
AROW = 64                    # f32 elems per adst row (256 B): adst[0:2]
SENT_A = NLOC                # adst sentinel row (-1e9)

_compiled = None  # (nc, NB_LO, NB_HI)


# --------------------------------------------------------------------------
# host-side graph preprocessing (pure index manipulation)
# --------------------------------------------------------------------------

def _wrap_idx(flat):
    """[n] -> [128, n/16] int16 wrapped + 8x replicated layout for dma_gather."""
    w = np.asarray(flat, np.int16).reshape(-1, 16).T
    return np.tile(w, (8, 1))


def _preprocess(edge_index):
    src = np.concatenate([edge_index[0].astype(np.int64), np.arange(N, dtype=np.int64)])
    dst = np.concatenate([edge_index[1].astype(np.int64), np.arange(N, dtype=np.int64)])
    order = np.argsort(dst, kind="stable")
    src, dst = src[order], dst[order]

    stg = (dst // NST_NODES).astype(np.int64)          # global supertile id, sorted
    starts = np.searchsorted(stg, np.arange(S_GLOBAL))
    ends = np.searchsorted(stg, np.arange(S_GLOBAL), side="right")
    lo_mask = src < V_HALF
    n_lo = np.array([int(lo_mask[a:b].sum()) for a, b in zip(starts, ends)])
    n_hi = (ends - starts) - n_lo
    nb_lo = int(np.ceil(n_lo.max() / 128))
    nb_hi = int(np.ceil(n_hi.max() / 128))
    cap_lo, cap_hi = nb_lo * 128, nb_hi * 128
    nb = nb_lo + nb_hi
    cap = cap_lo + cap_hi

    ilo = np.zeros((N_CORES, S, 128, cap_lo // 16), np.int16)
    ihi = np.zeros((N_CORES, S, 128, cap_hi // 16), np.int16)
    ia = np.zeros((N_CORES, S, 128, cap // 16), np.int16)
    slot = np.zeros((N_CORES, S, 128, nb), np.float32)

    for g in range(S_GLOBAL):
        k, t = divmod(g, S)
        a, b = starts[g], ends[g]
        s_src, s_dst = src[a:b], dst[a:b]
        m = s_src < V_HALF
        src_lo, dst_lo = s_src[m], s_dst[m]
        src_hi, dst_hi = s_src[~m], s_dst[~m]

        i_lo = np.zeros(cap_lo, np.int64)
        i_lo[: len(src_lo)] = src_lo
        i_hi = np.zeros(cap_hi, np.int64)
        i_hi[: len(src_hi)] = src_hi - V_HALF

        d_all = np.full(cap, SENT_A, np.int64)
        d_all[: len(dst_lo)] = dst_lo - NLOC * k
        d_all[cap_lo : cap_lo + len(dst_hi)] = dst_hi - NLOC * k

        sl = np.full(cap, 127, np.int64)
        sl[: len(dst_lo)] = dst_lo - NST_NODES * g
        sl[cap_lo : cap_lo + len(dst_hi)] = dst_hi - NST_NODES * g

        ilo[k, t] = _wrap_idx(i_lo)
        ihi[k, t] = _wrap_idx(i_hi)
        ia[k, t] = _wrap_idx(d_all)
        slot[k, t] = sl.reshape(nb, 128).T.astype(np.float32)

    return ilo, ihi, ia, slot, nb_lo, nb_hi


# --------------------------------------------------------------------------
# device program
# --------------------------------------------------------------------------

def _build_program(nb_lo, nb_hi, phases=7, elevel=3, repeats=1):
    import concourse.bass as bass
    import concourse.bacc as bacc
    import concourse.mybir as mybir
    from concourse import library_config
    from concourse.masks import make_identity
    from concourse.tile import TileContext

    f32 = mybir.dt.float32
    bf16 = mybir.dt.bfloat16
    i16 = mybir.dt.int16
    i32 = mybir.dt.int32
    Alu = mybir.AluOpType
    Act = mybir.ActivationFunctionType

    nb = nb_lo + nb_hi
    cap_lo, cap_hi, cap = nb_lo * 128, nb_hi * 128, (nb_lo + nb_hi) * 128

    nc = bacc.Bacc("TRN2", target_bir_lowering=False, debug=False, num_devices=N_CORES, num_swdge_queues=4)

    # ---- I/O
    x_in = nc.declare_dram_parameter("x", [N, FIN], f32, isOutput=False)
    x_own_in = nc.declare_dram_parameter("x_own", [NLOC, FIN], f32, isOutput=False)
    w1_in = nc.declare_dram_parameter("W1", [HC, FIN], f32, isOutput=False)
    w2_in = nc.declare_dram_parameter("W2", [HC, C], f32, isOutput=False)
    att1_in = nc.declare_dram_parameter("att1", [1, H, 2 * C], f32, isOutput=False)
    att2_in = nc.declare_dram_parameter("att2", [1, H, 2 * C], f32, isOutput=False)
    b1_in = nc.declare_dram_parameter("b1", [C], f32, isOutput=False)
    b2_in = nc.declare_dram_parameter("b2", [C], f32, isOutput=False)
    ilo_in = nc.declare_dram_parameter("ilo", [S, 128, cap_lo // 16], i16, isOutput=False)
    ihi_in = nc.declare_dram_parameter("ihi", [S, 128, cap_hi // 16], i16, isOutput=False)
    ia_in = nc.declare_dram_parameter("ia", [S, 128, cap // 16], i16, isOutput=False)
    slot_in = nc.declare_dram_parameter("slot", [S, 128, nb], f32, isOutput=False)
    out_ext = nc.declare_dram_parameter("out", [NLOC, C], f32, isOutput=True)

    # ---- internal DRAM
    table = nc.dram_tensor("table_d", [N, TROW], bf16)             # 25.6 MB
    adst_tbl = nc.dram_tensor("adst_d", [NLOC + 1, AROW], f32)     # 1.6 MB
    ar_in = nc.dram_tensor("ar_in_d", [1, H], f32)
    ar_out = nc.dram_tensor("ar_out_d", [1, H], f32, addr_space="Shared")
    ag_in = nc.dram_tensor("ag_in_d", [NLOC, C], bf16)
    act_full = nc.dram_tensor("act_full_d", [N, C], bf16, addr_space="Shared")

    NT_A = N // 128  # 391 full tiles (one partial handled below); N = 390*128 + 80
    nt_a_full, rem_a = divmod(N, 128)

    with TileContext(nc) as tc:
        with (
            tc.tile_pool(name="const", bufs=1) as cpool,
            tc.tile_pool(name="bld", bufs=3) as bld,
            tc.tile_pool(name="bldp", bufs=2, space="PSUM") as bldp,
            tc.tile_pool(name="gat", bufs=2) as gat,
            tc.tile_pool(name="edge", bufs=2) as edge,
            tc.tile_pool(name="up", bufs=2, space="PSUM") as upool,
            tc.tile_pool(name="fix", bufs=2) as fix,
        ):
            nc.gpsimd.load_library(library_config.mlp)

            # ============ constants ============
            ident = cpool.tile([128, 128], bf16)
            make_identity(nc, ident[:])
            ident_f = cpool.tile([128, 128], f32)
            make_identity(nc, ident_f[:])
            iota_i = cpool.tile([128, 128], i32)
            nc.gpsimd.iota(iota_i[:], pattern=[[1, 128]], base=0, channel_multiplier=0)
            iota_b = cpool.tile([128, 128], bf16)
            nc.vector.tensor_copy(out=iota_b[:], in_=iota_i[:])
            ones_row = cpool.tile([1, 128], f32)
            nc.vector.memset(ones_row[:], 1.0)
            ones_col = cpool.tile([128, 1], f32)
            nc.vector.memset(ones_col[:], 1.0)

            # all idx/slot arrays, resident (layer-independent)
            ilo_all = cpool.tile([128, S, cap_lo // 16], i16)
            ihi_all = cpool.tile([128, S, cap_hi // 16], i16)
            ia_all = cpool.tile([128, S, cap // 16], i16)
            nc.sync.dma_start(out=ilo_all[:], in_=ilo_in.ap().rearrange("s p w -> p s w"))
            nc.sync.dma_start(out=ihi_all[:], in_=ihi_in.ap().rearrange("s p w -> p s w"))
            nc.sync.dma_start(out=ia_all[:], in_=ia_in.ap().rearrange("s p w -> p s w"))
            slot_f = cpool.tile([128, S, nb], f32)
            nc.sync.dma_start(out=slot_f[:], in_=slot_in.ap().rearrange("s p w -> p s w"))
            slot_b = cpool.tile([128, S, nb], bf16)
            nc.vector.tensor_copy(out=slot_b[:], in_=slot_f[:])

            # bias broadcast tiles
            b1b = cpool.tile([128, C], f32)
            nc.sync.dma_start(out=b1b[:], in_=b1_in.ap().partition_broadcast(128))
            b2b = cpool.tile([128, C], f32)
            nc.sync.dma_start(out=b2b[:], in_=b2_in.ap().partition_broadcast(128))

            # sentinel row of adst table
            sent_t = cpool.tile([1, AROW], f32)
            nc.vector.memset(sent_t[:], -1.0e9)
            nc.sync.dma_start(out=adst_tbl[SENT_A : SENT_A + 1, :], in_=sent_t[:])

            # ---- R matrices: R = [W^T | Vsrc' | Vdst']  (bf16, [K=fin, 132])
            def build_R(w_dram, att_dram, kdim):
                wt = bld.tile([128, kdim], f32, tag="wld")
                nc.sync.dma_start(out=wt[:, :], in_=w_dram[:])          # [HC, kdim]
                wb = bld.tile([128, kdim], bf16, tag="wldb")
                nc.vector.tensor_copy(out=wb[:], in_=wt[:])
                vsd = cpool.tile([128, 4], f32, tag="vsd")  # cols 0:2 Vsrc, 2:4 Vdst
                nc.vector.memset(vsd[:], 0.0)
                # Vsrc[h*C+c, h] = att[0,h,C+c] ; Vdst[h*C+c, h] = att[0,h,c]
                for h in range(H):
                    nc.sync.dma_start(
                        out=vsd[h * C : (h + 1) * C, h : h + 1],
                        in_=att_dram[0:1, h, C : 2 * C].rearrange("o c -> c o"),
                    )
                    nc.sync.dma_start(
                        out=vsd[h * C : (h + 1) * C, 2 + h : 3 + h],
                        in_=att_dram[0:1, h, 0:C].rearrange("o c -> c o"),
                    )
                vsdb = cpool.tile([128, 4], bf16, tag="vsdb")
                nc.vector.tensor_copy(out=vsdb[:], in_=vsd[:])
                r_ps = bldp.tile([128, 132], f32, tag="mps", space="PSUM")
                # W^T via PE transpose (bf16): out[kdim, HC]
                nc.tensor.transpose(out=r_ps[:kdim, 0:128].bitcast(bf16)[:, 0:128], in_=wb[:, :], identity=ident[:])
                # Vsrc' / Vdst' = W^T @ V : lhsT=W [HC, kdim], rhs=V [HC, 2]
                nc.tensor.matmul(out=r_ps[:kdim, 128:132], lhsT=wb[:, :], rhs=vsdb[:, :], start=True, stop=True)
                r_sb = cpool.tile([128, 132], bf16, tag=f"R{kdim}")
                nc.vector.tensor_copy(out=r_sb[:kdim, 0:128], in_=r_ps[:kdim, 0:128].bitcast(bf16)[:, 0:128])
                nc.vector.tensor_copy(out=r_sb[:kdim, 128:132], in_=r_ps[:kdim, 128:132])
                return r_sb

            R1 = build_R(w1_in, att1_in, FIN)
            R2 = build_R(w2_in, att2_in, C)

            # resident state
            U_sb = cpool.tile([128, S, HC], f32)        # aggregation output per layer
            act_sb = cpool.tile([128, S, C], bf16)      # layer-1 activations (own nodes)
            gacc = cpool.tile([128, H], f32)

            # ============ helper: table build (phase A, replicated) ============
            def build_table_A(layer):
                kdim = FIN if layer == 1 else C
                R = R1 if layer == 1 else R2
                n_tiles = nt_a_full + (1 if rem_a else 0)
                for i in range(n_tiles):
                    r0 = i * 128
                    rows = 128 if i < nt_a_full else rem_a
                    if layer == 1:
                        xf = bld.tile([128, kdim], f32, tag="xaf")
                        nc.sync.dma_start(out=xf[:rows, :], in_=x_in[r0 : r0 + rows, :])
                        xt_ps = bldp.tile([128, 128], f32, tag="xtp", space="PSUM")
                        nc.tensor.transpose(out=xt_ps[:kdim, :rows], in_=xf[:rows, :], identity=ident_f[:rows, :rows])
                    else:
                        xb = bld.tile([128, kdim], bf16, tag="xa")
                        nc.sync.dma_start(out=xb[:rows, :], in_=act_full[r0 : r0 + rows, :])
                        xt_ps = bldp.tile([128, 128], bf16, tag="xtpb", space="PSUM")
                        nc.tensor.transpose(out=xt_ps[:kdim, :rows], in_=xb[:rows, :], identity=ident[:rows, :rows])
                    xt = bld.tile([128, 128], bf16, tag="xt")
                    nc.vector.tensor_copy(out=xt[:kdim, :rows], in_=xt_ps[:kdim, :rows])
                    h_ps = bldp.tile([128, 132], f32, tag="mps", space="PSUM")
                    nc.tensor.matmul(out=h_ps[:rows, :], lhsT=xt[:kdim, :rows], rhs=R[:kdim, :], start=True, stop=True)
                    trow = bld.tile([128, TROW], bf16, tag="trow")
                    nc.vector.memset(trow[:], 0.0)
                    nc.vector.tensor_copy(out=trow[:rows, 0:HC], in_=h_ps[:rows, 0:HC])
                    nc.vector.tensor_copy(out=trow[:rows, HC : HC + 2], in_=h_ps[:rows, 128:130])
                    nc.sync.dma_start(out=table[r0 : r0 + rows, :], in_=trow[:rows, :])

            # ============ helper: adst build (phase B, own nodes) ============
            def build_adst(layer):
                kdim = FIN if layer == 1 else C
                R = R1 if layer == 1 else R2
                for t in range(S):
                    r0 = t * NST_NODES
                    if layer == 1:
                        xf = bld.tile([128, kdim], f32, tag="xbf_b")
                        nc.sync.dma_start(out=xf[:NST_NODES, :], in_=x_own_in[r0 : r0 + NST_NODES, :])
                        xt_ps = bldp.tile([128, 128], f32, tag="xtp", space="PSUM")
                        nc.tensor.transpose(
                            out=xt_ps[:kdim, :NST_NODES], in_=xf[:NST_NODES, :], identity=ident_f[:NST_NODES, :NST_NODES]
                        )
                    else:
                        xb = bld.tile([128, kdim], bf16, tag="xb_b")
                        nc.vector.tensor_copy(out=xb[:NST_NODES, :], in_=act_sb[0:NST_NODES, t, :])
                        xt_ps = bldp.tile([128, 128], bf16, tag="xtpb", space="PSUM")
                        nc.tensor.transpose(
                            out=xt_ps[:kdim, :NST_NODES], in_=xb[:NST_NODES, :], identity=ident[:NST_NODES, :NST_NODES]
                        )
                    xt = bld.tile([128, 128], bf16, tag="xt_b")
                    nc.vector.tensor_copy(out=xt[:kdim, :NST_NODES], in_=xt_ps[:kdim, :NST_NODES])
                    a_ps = bldp.tile([128, 2], f32, tag="mps", space="PSUM")
                    nc.tensor.matmul(
                        out=a_ps[:NST_NODES, :], lhsT=xt[:kdim, :NST_NODES], rhs=R[:kdim, 130:132], start=True, stop=True
                    )
                    a_sb = bld.tile([128, AROW], f32, tag="asb_b")
                    nc.vector.memset(a_sb[:], 0.0)
                    nc.vector.tensor_copy(out=a_sb[:NST_NODES, 0:2], in_=a_ps[:NST_NODES, :])
                    nc.sync.dma_start(out=adst_tbl[r0 : r0 + NST_NODES, :], in_=a_sb[:NST_NODES, :])

            # ============ helper: edge pass ============
            GMAX = 8  # max 128-blocks (1024 indices) per dma_gather call

            def chunked_gather(dst, blk0, nblk, src, idx_all, t, qctr, row):
                c = 0
                while c < nblk:
                    n = min(GMAX, nblk - c)
                    nc.gpsimd.dma_gather(
                        dst[:, blk0 + c : blk0 + c + n, :], src,
                        idx_all[:, t, (c * 8) : (c + n) * 8],
                        n * 128, n * 128, row, queue_num=qctr[0] % 4,
                    )
                    qctr[0] += 1
                    c += n

            def edge_pass(layer):
                nc.vector.memset(gacc[:], 0.0)
                qctr = [0]
                for t in range(S):
                    g_t = gat.tile([128, nb, TROW], bf16, tag="G")
                    chunked_gather(g_t, 0, nb_lo, table[0:V_HALF, :], ilo_all, t, qctr, TROW)
                    chunked_gather(g_t, nb_lo, nb_hi, table[V_HALF:N, :], ihi_all, t, qctr, TROW)
                    a_t = gat.tile([128, nb, AROW], f32, tag="A")
                    chunked_gather(a_t, 0, nb, adst_tbl[:], ia_all, t, qctr, AROW)
                    if elevel < 2:
                        nc.vector.tensor_copy(out=U_sb[:, t, 0:AROW], in_=a_t[:, 0, :])
                        continue
                    # scores: s = asrc + adst  -> lrelu -> exp (+gsum accum)
                    asrc_f = edge.tile([128, nb, 2], f32, tag="asrc")
                    nc.vector.tensor_copy(out=asrc_f[:], in_=g_t[:, :, HC : HC + 2])
                    s_t = edge.tile([128, nb, 2], f32, tag="s")
                    nc.vector.tensor_tensor(out=s_t[:], in0=asrc_f[:], in1=a_t[:, :, 0:2], op=Alu.add)
                    sl_t = edge.tile([128, nb, 2], f32, tag="sl")
                    neg_t = edge.tile([128, nb, 2], f32, tag="neg")
                    nc.vector.tensor_scalar(
                        out=neg_t[:], in0=s_t[:], scalar1=0.0, scalar2=0.2,
                        op0=Alu.min, op1=Alu.mult,
                    )
                    nc.vector.tensor_scalar(out=sl_t[:], in0=s_t[:], scalar1=0.0, scalar2=None, op0=Alu.max)
                    nc.vector.tensor_tensor(out=sl_t[:], in0=sl_t[:], in1=neg_t[:], op=Alu.add)
                    w_f = edge.tile([128, nb, 2], f32, tag="wf")
                    acc_t = edge.tile([128, H], f32, tag="acc")
                    for h in range(H):
                        nc.scalar.activation(
                            out=w_f[:, :, h], in_=sl_t[:, :, h], func=Act.Exp,
                            accum_out=acc_t[:, h : h + 1],
                        )
                    nc.vector.tensor_tensor(out=gacc[:], in0=gacc[:], in1=acc_t[:], op=Alu.add)
                    w_b = edge.tile([128, nb, 2], bf16, tag="wb")
                    nc.vector.tensor_copy(out=w_b[:], in_=w_f[:])
                    # messages = h * w
                    msg = edge.tile([128, nb, HC], bf16, tag="msg")
                    for h in range(H):
                        nc.vector.tensor_tensor(
                            out=msg[:, :, h * C : (h + 1) * C],
                            in0=g_t[:, :, h * C : (h + 1) * C],
                            in1=w_b[:, :, h : h + 1].to_broadcast([128, nb, C]),
                            op=Alu.mult,
                        )
                    if elevel < 3:
                        nc.vector.tensor_copy(out=U_sb[:, t, :], in_=msg[:, 0, :])
                        continue
                    # one-hot segment-sum into PSUM
                    u_ps = upool.tile([128, HC], f32, tag="U", space="PSUM")
                    for j in range(nb):
                        oh = edge.tile([128, 128], bf16, tag="oh")
                        nc.vector.tensor_scalar(
                            out=oh[:], in0=iota_b[:],
                            scalar1=slot_f[:, t, j : j + 1], scalar2=None,
                            op0=Alu.is_equal,
                        )
                        nc.tensor.matmul(
                            out=u_ps[:], lhsT=oh[:], rhs=msg[:, j, :],
                            start=(j == 0), stop=(j == nb - 1),
                        )
                    nc.vector.tensor_copy(out=U_sb[:, t, :], in_=u_ps[:])

            # ============ helper: stats allreduce + fixup ============
            def stats_and_fixup(layer):
                # partition-reduce gacc -> [1, H]
                g_ps = bldp.tile([128, H], f32, tag="mps", space="PSUM")
                nc.tensor.matmul(out=g_ps[0:1, :], lhsT=ones_col[:], rhs=gacc[:], start=True, stop=True)
                g_sb = fix.tile([1, H], f32, tag="gsb")
                nc.vector.tensor_copy(out=g_sb[:], in_=g_ps[0:1, :])
                nc.sync.dma_start(out=ar_in[:], in_=g_sb[:])
                tc.strict_bb_all_engine_barrier()
                nc.gpsimd.collective_compute(
                    "AllReduce", mybir.AluOpType.add,
                    replica_groups=[list(range(N_CORES))],
                    ins=[ar_in[:]], outs=[ar_out[:]],
                )
                tg = fix.tile([1, H], f32, tag="tg")
                nc.sync.dma_start(out=tg[:], in_=ar_out[:])
                # broadcast to 128 partitions, then rT = 0.5 / (T + 1e-10)
                tb_ps = bldp.tile([128, H], f32, tag="mps", space="PSUM")
                nc.tensor.matmul(out=tb_ps[:], lhsT=ones_row[0:1, :], rhs=tg[:], start=True, stop=True)
                tb = fix.tile([128, H], f32, tag="tb")
                nc.vector.tensor_scalar(out=tb[:], in0=tb_ps[:], scalar1=1.0e-10, scalar2=None, op0=Alu.add)
                rt = fix.tile([128, H], f32, tag="rt")
                nc.vector.reciprocal(out=rt[:], in_=tb[:])
                nc.vector.tensor_scalar(out=rt[:], in0=rt[:], scalar1=0.5, scalar2=None, op0=Alu.mult)

                bias = b1b if layer == 1 else b2b
                for t in range(S):
                    m0 = fix.tile([128, C], f32, tag="m0")
                    nc.vector.tensor_scalar(
                        out=m0[:], in0=U_sb[:, t, 0:C], scalar1=rt[:, 0:1], scalar2=None, op0=Alu.mult
                    )
                    m1 = fix.tile([128, C], f32, tag="m1")
                    nc.vector.tensor_scalar(
                        out=m1[:], in0=U_sb[:, t, C:HC], scalar1=rt[:, 1:2], scalar2=None, op0=Alu.mult
                    )
                    nc.vector.tensor_tensor(out=m0[:], in0=m0[:], in1=m1[:], op=Alu.add)
                    nc.vector.tensor_tensor(out=m0[:], in0=m0[:], in1=bias[:], op=Alu.add)
                    if layer == 1:
                        # act = relu(out1), keep bf16 copy + stage for AllGather
                        nc.vector.tensor_scalar(out=m0[:], in0=m0[:], scalar1=0.0, scalar2=None, op0=Alu.max)
                        nc.vector.tensor_copy(out=act_sb[:, t, :], in_=m0[:])
                        nc.sync.dma_start(
                            out=ag_in[t * NST_NODES : (t + 1) * NST_NODES, :],
                            in_=act_sb[0:NST_NODES, t, :],
                        )
                    else:
                        nc.sync.dma_start(
                            out=out_ext[t * NST_NODES : (t + 1) * NST_NODES, :],
                            in_=m0[:NST_NODES, :],
                        )

            # ============ main sequence ============
            for _rep in range(repeats):
                if _rep > 0:
                    tc.strict_bb_all_engine_barrier()
                if phases >= 1:
                    build_table_A(1)
                    build_adst(1)
                if phases >= 2:
                    tc.strict_bb_all_engine_barrier()
                    edge_pass(1)
                if phases >= 3:
                    stats_and_fixup(1)
                if phases >= 4:
                    tc.strict_bb_all_engine_barrier()
                    nc.gpsimd.collective_compute(
                        "AllGather", mybir.AluOpType.bypass,
                        replica_groups=[list(range(N_CORES))],
                        ins=[ag_in[:]], outs=[act_full[:]],
                    )
                if phases >= 5:
                    tc.strict_bb_all_engine_barrier()
                    build_table_A(2)
                    build_adst(2)
                if phases >= 6:
                    tc.strict_bb_all_engine_barrier()
                    edge_pass(2)
                if phases >= 7:
                    stats_and_fixup(2)

    nc.compile()
    return nc


# --------------------------------------------------------------------------
# entry point
# --------------------------------------------------------------------------

def kernel(x, edge_index, W1, att1, b1, W2, att2, b2):
    global _compiled
    from concourse.bass_utils import run_bass_kernel_spmd

    x = np.asarray(x, np.float32)
    W1 = np.asarray(W1, np.float32)
    W2 = np.asarray(W2, np.float32)
    att1 = np.asarray(att1, np.float32)
    att2 = np.asarray(att2, np.float32)
    b1 = np.asarray(b1, np.float32)
    b2 = np.asarray(b2, np.float32)

    ilo, ihi, ia, slot, nb_lo, nb_hi = _preprocess(np.asarray(edge_index))

    if _compiled is None or _compiled[1] != (nb_lo, nb_hi):
        nc = _build_program(nb_lo, nb_hi)
        _compiled = (nc, (nb_lo, nb_hi))
    nc = _compiled[0]

    in_maps = []
    for k in range(N_CORES):
        in_maps.append({
            "x": x, "x_own": x[k * NLOC : (k + 1) * NLOC],
            "W1": W1, "W2": W2, "att1": att1, "att2": att2, "b1": b1, "b2": b2,
            "ilo": ilo[k], "ihi": ihi[k], "ia": ia[k], "slot": slot[k],
        })
    res = run_bass_kernel_spmd(nc, in_maps, list(range(N_CORES)))
    out = np.concatenate([res.results[k]["out"] for k in range(N_CORES)], axis=0)
    return out

